# revision 14
# baseline (speedup 1.0000x reference)
"""Trainium2 Bass kernel for nn_Block_50113678410401 (dense transformer block).

Strategy: data-parallel over the batch axis (B=8 -> 8 NeuronCores, one batch
element per core). All on-chip activations live in "layout A": feature axis on
SBUF partitions, token axis (T) on the free dimension, so no on-chip
transposes are needed (host pre-transposes x and post-transposes the output).

Per core:
  LN1 (stats via ones-matmul over partitions), per-head causal attention
  (no-max-sub exp softmax, denominator via ones-matmul, normalization via
  K=1 broadcast matmul + wide reciprocal), output projection + residual,
  BatchNorm over (B,C) with per-chunk cross-core AllReduces of (sum, sumsq)
  per T channel, LN2, FFN (C -> 4C -> relu -> C), residual, second BatchNorm.

Collective-latency hiding:
  * a tiny warm-up AllReduce at kernel start absorbs the first-collective
    ramp cost (~50us observed on the first op of a NEFF).
  * the LN2 affine of BN1(u1) is computed from LOCAL per-core stats only
    (the global BN1 scale enters the exact formula only through an eps/s^2
    term; substituting the local-sample estimate of s changes the result by
    ~1e-7 relative).  So the FFN starts immediately after the out-projection
    without waiting for the BN1 AllReduce; the global BN1 params are only
    needed for the residual affine consumed by mm2's residual-add, ~150us
    after the collective is triggered.
  * the FFN runs in token sub-chunks of 512/256/256 columns; each sub-chunk
    triggers its BN2 stats AllReduce when its mm2 finishes, so the final
    (tail) AllReduce covers only 256 columns and its finale is short.

All big matmuls run in bf16 with fp32 PSUM accumulation; statistics,
softmax, residuals and normalizations are fp32. Weights arrive host-pretiled
so every weight DMA is contiguous per partition.

LayerNorm/projection affine parameters are folded into the weights on the
host: wq' = diag(ln1_g) wq / sqrt(D) (q also carries 1/sqrt(D)), k-side bias
drops out of softmax by shift invariance, v-side bias is folded into the
output-projection bias, ln2 affine is folded into w1/b1.
"""

import numpy as np
import ml_dtypes

B, T, C, H, D = 8, 1024, 1536, 12, 128
F = 4 * C            # 6144
P = 128
CT = C // P          # 12 c-tiles
FT = F // P          # 48 f-tiles
ST = T // P          # 8 s-tiles
CH = 512             # matmul free-dim chunk
NCH = T // CH        # 2 chunks
EPS = 1e-5
NCORES = 8
NBC = B * C          # BatchNorm count over (B, C)

# FFN token sub-chunks (offset, width).  512-wide: the w1/w2 weight stream
# (18.9 MB per pass) needs the full pass duration to fit in HBM bandwidth;
# narrower chunks starve the PE on weight DMA.
FFN_SUB = [(0, 512), (512, 512)]

_PROG = None


def _build():
    import concourse.bass as bass
    import concourse.mybir as mybir
    import concourse.tile as tile
    from concourse import bacc
    from concourse.masks import make_upper_triangular

    fp32 = mybir.dt.float32
    bf16 = mybir.dt.bfloat16
    AF = mybir.ActivationFunctionType
    OP = mybir.AluOpType
    ts = bass.ts

    nc = bacc.Bacc("TRN2", target_bir_lowering=False, debug=False,
                   enable_asserts=True, num_devices=NCORES)

    # ---- DRAM I/O (weights host-pretiled for contiguous DMA) ----
    xT_d = nc.dram_tensor("xT", (C, T), fp32, kind="ExternalInput").ap()
    xbf_d = nc.dram_tensor("xbf", (C, T), bf16, kind="ExternalInput").ap()
    wq_d = nc.dram_tensor("wq", (H, P, CT, P), bf16, kind="ExternalInput").ap()
    wk_d = nc.dram_tensor("wk", (H, P, CT, P), bf16, kind="ExternalInput").ap()
    wv_d = nc.dram_tensor("wv", (C, C), bf16, kind="ExternalInput").ap()
    bq_d = nc.dram_tensor("bq", (P, H), fp32, kind="ExternalInput").ap()
    wo_d = nc.dram_tensor("wo", (CT, P, H, P), bf16, kind="ExternalInput").ap()
    bo_d = nc.dram_tensor("bo", (P, CT), fp32, kind="ExternalInput").ap()
    w1_d = nc.dram_tensor("w1", (FT, P, CT, P), bf16, kind="ExternalInput").ap()
    b1_d = nc.dram_tensor("b1", (P, FT), fp32, kind="ExternalInput").ap()
    w2_d = nc.dram_tensor("w2", (CT, P, FT, P), bf16, kind="ExternalInput").ap()
    b2_d = nc.dram_tensor("b2", (P, CT), fp32, kind="ExternalInput").ap()
    bn1g_d = nc.dram_tensor("bn1g", (1, T), fp32, kind="ExternalInput").ap()
    bn1b_d = nc.dram_tensor("bn1b", (1, T), fp32, kind="ExternalInput").ap()
    bn2g_d = nc.dram_tensor("bn2g", (1, T), fp32, kind="ExternalInput").ap()
    bn2b_d = nc.dram_tensor("bn2b", (1, T), fp32, kind="ExternalInput").ap()
    yT_d = nc.dram_tensor("yT", (C, T), fp32, kind="ExternalOutput").ap()

    with tile.TileContext(nc) as tc:
        with tc.tile_pool(name="const", bufs=1) as cpool, \
             tc.tile_pool(name="scratch", bufs=1) as spool, \
             tc.tile_pool(name="u1p", bufs=1) as u1pool, \
             tc.tile_pool(name="wstr", bufs=1) as wstream, \
             tc.tile_pool(name="ppw", bufs=6, space="PSUM") as ppw, \
             tc.tile_pool(name="pps", bufs=2, space="PSUM") as pps, \
             tc.tile_pool(name="dram", bufs=1, space="DRAM") as dpool:

            # ---- constants ----
            ones_bf = cpool.tile([P, 1], bf16, name="ones_bf")
            nc.vector.memset(ones_bf[:], 1.0)
            trimask = cpool.tile([P, P], bf16, name="trimask")
            make_upper_triangular(nc, trimask[:], val=1.0, diag=True)
            bq_sb = cpool.tile([P, H], fp32, name="bq_sb")
            nc.sync.dma_start(bq_sb[:], bq_d[:])
            bo_sb = cpool.tile([P, CT], fp32, name="bo_sb")
            nc.sync.dma_start(bo_sb[:], bo_d[:])
            b1_sb = cpool.tile([P, FT], fp32, name="b1_sb")
            nc.sync.dma_start(b1_sb[:], b1_d[:])
            b2_sb = cpool.tile([P, CT], fp32, name="b2_sb")
            nc.sync.dma_start(b2_sb[:], b2_d[:])

            # Warm-up AllReduce: the first collective of a NEFF pays a large
            # ramp cost (~50us observed); burn it on a 64B dummy during the
            # attention phase so the real BN AllReduces run at steady-state
            # latency.
            wrow = cpool.tile([1, 16], fp32, name="warm_cc_row")
            nc.vector.memset(wrow[:], 0.0)
            wcin = dpool.tile([1, 16], fp32, name="warm_cc_in")
            wcout = dpool.tile([1, 16], fp32, name="warm_cc_out")
            nc.gpsimd.dma_start(wcin[:], wrow[:])
            nc.gpsimd.collective_compute(
                "AllReduce", mybir.AluOpType.add,
                replica_groups=[list(range(NCORES))],
                ins=[wcin.opt()], outs=[wcout.opt()],
            )

            # PE clock warm-up: ~5us of junk matmuls during the initial
            # x DMA so the LN1 stats matmuls run at full clock.
            warm_ps = ppw.tile([P, P], fp32, tag="w", name="warm_ps")
            for _ in range(40):
                nc.tensor.matmul(warm_ps[:], trimask[:], trimask[:],
                                 start=True, stop=True)

            # ---- helpers ----
            def bcast_into(dst_ap, row_ap, name, n=CH, eng=None):
                """(1, n) fp32 SBUF row -> (P, n) via DRAM bounce on the given
                DMA-capable engine queue (default scalar)."""
                e = eng if eng is not None else nc.scalar
                dr = dpool.tile([1, n], fp32, name=f"{name}_dr")
                e.dma_start(dr[:], row_ap)
                e.dma_start(dst_ap, dr[:].to_broadcast((P, n)))

            ones1f = cpool.tile([1, P], fp32, name="ones1f")
            nc.vector.memset(ones1f[:], 1.0)

            def bc_mm_into(dst_ap, row_ap, name, n=CH):
                """(1, n) fp32 row -> (P, n) SBUF via K=1 matmul + copy.
                Higher PE cost than bcast_into but ~3x lower latency; used on
                norm-param critical paths where the PE is idle anyway."""
                ps = ppw.tile([P, CH], fp32, tag="w", name=f"{name}_ps")
                nc.tensor.matmul(ps[:, :n], ones1f[:], row_ap, start=True,
                                 stop=True)
                nc.scalar.copy(dst_ap, ps[:, :n])

            # Packed stat psum tile: row 0 accumulates sum, row 32 sumsq.
            def stat_tiles(name, cnt=NCH):
                return [pps.tile([P, CH], fp32, tag="st", bufs=2,
                                 name=f"{name}_{j}") for j in range(cnt)]

            def stats_chunk(src_ap, stp_j, first, last, is_bf16=False,
                            width=CH):
                """Ones-matmul partial sums of src chunk ((P,width)) and its
                square into packed stat rows."""
                if is_bf16:
                    cbf = src_ap
                else:
                    cbf_t = spool.tile([P, CH], bf16, tag="cast_bf", bufs=2,
                                       name="cbf")
                    nc.vector.tensor_copy(cbf_t[:, :width], src_ap)
                    cbf = cbf_t[:, :width]
                csq = spool.tile([P, CH], bf16, tag="cast_sq", bufs=2,
                                 name="csq")
                nc.scalar.square(csq[:, :width], src_ap)
                nc.tensor.matmul(stp_j[0:1, :width], ones_bf[:], cbf,
                                 start=first, stop=last)
                nc.tensor.matmul(stp_j[32:33, :width], ones_bf[:],
                                 csq[:, :width], start=first, stop=last)

            def allreduce_chunk(pool, stp_j, name, width=CH):
                """AllReduce-add this chunk's packed (sum, sumsq) across
                cores. Returns the (1, 2*width) (local, global) rows."""
                loc = pool.tile([1, 2 * CH], fp32, tag="arloc", bufs=1,
                                name=f"{name}_loc")
                nc.scalar.copy(loc[:, 0:width], stp_j[0:1, :width])
                nc.scalar.copy(loc[:, width:2 * width], stp_j[32:33, :width])
                cin = dpool.tile([1, 2 * width], fp32, name=f"{name}_cin")
                cout = dpool.tile([1, 2 * width], fp32, name=f"{name}_cout")
                nc.gpsimd.dma_start(cin[:], loc[:, :2 * width])
                nc.gpsimd.collective_compute(
                    "AllReduce", mybir.AluOpType.add,
                    replica_groups=[list(range(NCORES))],
                    ins=[cin.opt()], outs=[cout.opt()],
                )
                glob = pool.tile([1, 2 * CH], fp32, tag="arglob", bufs=1,
                                 name=f"{name}_glob")
                nc.gpsimd.dma_start(glob[:, :2 * width], cout[:])
                return loc, glob

            def norm_params_chunk(pool, s1_ap, s2_ap, count, name,
                                  g_row_sl=None, b_row_sl=None,
                                  sc_tag="nsc", bi_tag="nbi", bc_pool=None,
                                  bc_bufs=1, via_dma=False, width=CH):
                """Per-chunk normalization params, computed at row level
                (single-lane, cheap custom-DVE reciprocal), then broadcast
                to (P, width). Returns (sc_bc, bi_bc)."""
                m = pool.tile([1, CH], fp32, tag="rm", bufs=1,
                              name=f"{name}_m")
                nc.vector.tensor_scalar_mul(m[:, :width], s1_ap, 1.0 / count)
                v = pool.tile([1, CH], fp32, tag="rv", bufs=1,
                              name=f"{name}_v")
                nc.vector.tensor_scalar_mul(v[:, :width], s2_ap, 1.0 / count)
                bias = pool.tile([1, CH], fp32, tag="rb", bufs=1,
                                 name=f"{name}_brow")
                nc.vector.tensor_mul(bias[:, :width], m[:, :width],
                                     m[:, :width])
                nc.vector.tensor_sub(v[:, :width], v[:, :width],
                                     bias[:, :width])
                nc.vector.tensor_scalar_add(v[:, :width], v[:, :width], EPS)
                nc.scalar.sqrt(v[:, :width], v[:, :width])
                scale = pool.tile([1, CH], fp32, tag="rs", bufs=1,
                                  name=f"{name}_srow")
                if g_row_sl is not None:
                    rc = pool.tile([1, CH], fp32, tag="rr", bufs=1,
                                   name=f"{name}_rc")
                    nc.vector.reciprocal_approx_fast(rc[:, :width],
                                                     v[:, :width])
                    nc.vector.tensor_mul(scale[:, :width], rc[:, :width],
                                         g_row_sl)
                else:
                    nc.vector.reciprocal_approx_fast(scale[:, :width],
                                                     v[:, :width])
                nc.vector.tensor_mul(bias[:, :width], m[:, :width],
                                     scale[:, :width])
                nc.vector.tensor_scalar_mul(bias[:, :width], bias[:, :width],
                                            -1.0)
                if b_row_sl is not None:
                    nc.vector.tensor_add(bias[:, :width], bias[:, :width],
                                         b_row_sl)
                bpool = bc_pool if bc_pool is not None else pool
                sc_bc = bpool.tile([P, CH], fp32, tag=sc_tag, bufs=bc_bufs,
                                   name=f"{name}_scbc")
                bi_bc = bpool.tile([P, CH], fp32, tag=bi_tag, bufs=bc_bufs,
                                   name=f"{name}_bibc")
                if via_dma:
                    bcast_into(sc_bc[:, :width], scale[:, :width],
                               f"{name}_sc", n=width, eng=nc.gpsimd)
                    bcast_into(bi_bc[:, :width], bias[:, :width],
                               f"{name}_bi", n=width, eng=nc.gpsimd)
                else:
                    bc_mm_into(sc_bc[:, :width], scale[:, :width],
                               f"{name}_sc", n=width)
                    bc_mm_into(bi_bc[:, :width], bias[:, :width],
                               f"{name}_bi", n=width)
                return sc_bc, bi_bc

            def affine_chunk(dst_ap, src_ap, sc_ap, bi_ap, width=CH):
                """dst = src * sc + bi on one (P, width) chunk."""
                tmp = spool.tile([P, CH], fp32, tag="ntmp", bufs=2,
                                 name="ntmp")
                nc.vector.tensor_mul(tmp[:, :width], src_ap, sc_ap)
                nc.vector.tensor_add(dst_ap, tmp[:, :width], bi_ap)

            u1 = []     # created at phase 4 (first use)
            o_nrm = []  # created at phase 3
            h2pool = None  # right-side pool, alloc'd after phase 1's peak
            h2T = []

            if True:
              with tc.tile_pool(name="onrm", bufs=1) as opool:
                with tc.tile_pool(name="hT", bufs=1) as hpool:
                    # hT split per column chunk (separate tiles) so chunk-0
                    # stats/params/affines/V start after only half the x DMA
                    # (dependency tracking is tile-granular).
                    hT = [[hpool.tile([P, CH], bf16, tag=f"h{k}_{j}",
                                      name=f"hT_{k}_{j}")
                           for j in range(NCH)] for k in range(CT)]

                    def hT_s(k, s):
                        """(P, P) slice of hT for s-tile s."""
                        j = (s * P) // CH
                        return hT[k][j][:, s * P - j * CH:(s + 1) * P - j * CH]

                    # ================= Phase 1: LN1 =================
                    with tc.tile_pool(name="vall2", bufs=1) as vpool:
                        Vall = [vpool.tile([P, C], bf16, tag=f"v{s}",
                                           name=f"V_{s}") for s in range(ST)]
                        with tc.tile_pool(name="p1", bufs=1) as p1:
                            stp = stat_tiles("ln1")
                            for j in range(NCH):
                                sl = slice(j * CH, (j + 1) * CH)
                                for k in range(CT):
                                    nc.sync.dma_start(hT[k][j][:],
                                                      xbf_d[ts(k, P), sl])
                                for k in range(CT):
                                    stats_chunk(hT[k][j][:], stp[j], k == 0,
                                                k == CT - 1, is_bf16=True)
                            ln1p = []
                            for j in range(NCH):
                                ln1p.append(norm_params_chunk(
                                    p1, stp[j][0:1, :], stp[j][32:33, :], C,
                                    f"ln1_{j}", bc_bufs=2))
                            # chunk-0 affines, V for the first four s-tiles,
                            # then chunk-1 affines, V for the rest.
                            sc0, bi0 = ln1p[0][0], ln1p[0][1]
                            for k in range(CT):
                                affine_chunk(hT[k][0][:], hT[k][0][:],
                                             sc0[:], bi0[:])

                            with tc.tile_pool(name="wv", bufs=1) as wvpool:
                                def v_pass(s_lo, s_hi, tag):
                                    for n in range(C // CH):
                                        wv_sb = []
                                        for k in range(CT):
                                            wvk = wvpool.tile(
                                                [P, CH], bf16, tag=f"wv{k}",
                                                bufs=2,
                                                name=f"wv_{k}_{n}_{tag}")
                                            nc.sync.dma_start(
                                                wvk[:],
                                                wv_d[ts(k, P), ts(n, CH)])
                                            wv_sb.append(wvk)
                                        for s in range(s_lo, s_hi):
                                            vps = ppw.tile(
                                                [P, CH], fp32, tag="w",
                                                name=f"v_ps_{s}_{n}")
                                            for k in range(CT):
                                                nc.tensor.matmul(
                                                    vps[:], hT_s(k, s),
                                                    wv_sb[k][:],
                                                    start=(k == 0),
                                                    stop=(k == CT - 1))
                                            nc.scalar.copy(
                                                Vall[s][:, ts(n, CH)], vps[:])

                                v_pass(0, ST // 2, "a")
                                sc1, bi1 = ln1p[1][0], ln1p[1][1]
                                for k in range(CT):
                                    affine_chunk(hT[k][1][:], hT[k][1][:],
                                                 sc1[:], bi1[:])
                                v_pass(ST // 2, ST, "b")

                        # ============ Phase 3: per-head attention ==========
                        with tc.tile_pool(name="p3", bufs=1) as p3:
                            for h in range(H):
                                o_nrm.append(opool.tile(
                                    [P, T], bf16, tag=f"o{h}", name=f"on_{h}"))

                            def qk_proj(h):
                                """Q/K projection matmuls + PSUM->SBUF
                                copies for head h."""
                                wqh = p3.tile([P, CT, P], bf16, tag="wqh",
                                              bufs=2, name=f"wqh_{h}")
                                nc.sync.dma_start(wqh[:], wq_d[h])
                                wkh = p3.tile([P, CT, P], bf16, tag="wkh",
                                              bufs=2, name=f"wkh_{h}")
                                nc.sync.dma_start(wkh[:], wk_d[h])
                                qT = p3.tile([P, T], bf16, tag="qT", bufs=2,
                                             name=f"qT_{h}")
                                kT = p3.tile([P, T], bf16, tag="kT", bufs=2,
                                             name=f"kT_{h}")
                                # k-outer, j-inner: consecutive matmuls
                                # share the same stationary weight tile
                                qps = [ppw.tile([P, CH], fp32, tag="w",
                                                name=f"q_ps_{h}_{j}")
                                       for j in range(NCH)]
                                kps = [ppw.tile([P, CH], fp32, tag="w",
                                                name=f"k_ps_{h}_{j}")
                                       for j in range(NCH)]
                                for k in range(CT):
                                    for j in range(NCH):
                                        nc.tensor.matmul(qps[j][:],
                                                         wqh[:, k, :],
                                                         hT[k][j][:],
                                                         start=(k == 0),
                                                         stop=(k == CT - 1))
                                for k in range(CT):
                                    for j in range(NCH):
                                        nc.tensor.matmul(kps[j][:],
                                                         wkh[:, k, :],
                                                         hT[k][j][:],
                                                         start=(k == 0),
                                                         stop=(k == CT - 1))
                                for j in range(NCH):
                                    sl = slice(j * CH, (j + 1) * CH)
                                    nc.scalar.activation(qT[:, sl], qps[j][:],
                                                         AF.Identity,
                                                         bias=bq_sb[:, h:h + 1],
                                                         scale=1.0)
                                    nc.scalar.copy(kT[:, sl], kps[j][:])
                                return qT, kT

                            def scores_exp(h, qT, kT):
                                """Causal scores + exp for head h (s-tile
                                covers t >= s*P)."""
                                aT = []
                                for s in range(ST):
                                    at = p3.tile([P, T - s * P], bf16,
                                                 tag=f"a{s}", bufs=1,
                                                 name=f"aT_{h}_{s}")
                                    aT.append(at)
                                    for j in range(NCH):
                                        lo = max(j * CH, s * P)
                                        hi = (j + 1) * CH
                                        if lo >= hi:
                                            continue
                                        sps = ppw.tile([P, CH], fp32, tag="w",
                                                       name=f"s_ps_{h}_{s}_{j}")
                                        nc.tensor.matmul(sps[:, :hi - lo],
                                                         kT[:, ts(s, P)],
                                                         qT[:, lo:hi],
                                                         start=True, stop=True)
                                        nc.scalar.activation(
                                            at[:, lo - s * P:hi - s * P],
                                            sps[:, :hi - lo], AF.Exp)
                                    nc.vector.tensor_mul(at[:, 0:P],
                                                         at[:, 0:P],
                                                         trimask[:])
                                return aT

                            def den_av(h, aT):
                                """Denominators, 1/den broadcast, attn @ V,
                                normalize into o_nrm[h]."""
                                den_ps = pps.tile([P, CH], fp32, tag="st",
                                                  bufs=2, name=f"dn_{h}")
                                for j in range(NCH):
                                    r0 = 32 * j
                                    smax = min(ST, 4 * (j + 1))
                                    for s in range(smax):
                                        lo = max(0, s * P - j * CH)
                                        nc.tensor.matmul(
                                            den_ps[r0:r0 + 1, lo:CH],
                                            ones_bf[:],
                                            aT[s][:, j * CH + lo - s * P:
                                                  (j + 1) * CH - s * P],
                                            start=(s == 0), stop=(s == smax - 1))
                                r_bc = p3.tile([P, T], fp32, tag="rbc", bufs=2,
                                               name=f"rbc_{h}")
                                for j in range(NCH):
                                    dj = p3.tile([1, CH], fp32, tag="den",
                                                 bufs=2, name=f"den_{h}_{j}")
                                    nc.scalar.copy(
                                        dj[:], den_ps[32 * j:32 * j + 1, :])
                                    rj = p3.tile([1, CH], fp32, tag="rrow",
                                                 bufs=2, name=f"rr_{h}_{j}")
                                    nc.vector.reciprocal_approx_fast(
                                        rj[:], dj[:])
                                    bcast_into(r_bc[:, j * CH:(j + 1) * CH],
                                               rj[:], f"rbc_{h}_{j}")
                                for j in range(NCH):
                                    smax = min(ST, 4 * (j + 1))
                                    ops_ = ppw.tile([P, CH], fp32, tag="w",
                                                    name=f"o_ps_{h}_{j}")
                                    for s in range(smax):
                                        lo = max(0, s * P - j * CH)
                                        nc.tensor.matmul(
                                            ops_[:, lo:CH],
                                            Vall[s][:, ts(h, P)],
                                            aT[s][:, j * CH + lo - s * P:
                                                  (j + 1) * CH - s * P],
                                            start=(s == 0), stop=(s == smax - 1))
                                    sl = slice(j * CH, (j + 1) * CH)
                                    nc.vector.tensor_mul(o_nrm[h][:, sl],
                                                         ops_[:], r_bc[:, sl])

                            # Software pipeline: head h-1's denominator /
                            # attn@V matmuls are emitted AFTER head h's Q/K
                            # projection matmuls, so the PE never waits
                            # head-of-line on head h-1's exp (ACT) -- it does
                            # head h's projections meanwhile; den/attnV then
                            # dispatch with their inputs long ready.
                            prev = None
                            for h in range(H):
                                qkt = qk_proj(h)
                                if prev is not None:
                                    den_av(prev[0], prev[1])
                                aT = scores_exp(h, *qkt)
                                prev = (h, aT)
                            den_av(prev[0], prev[1])

                # hT closed; Phase 4: out-proj + residual + BN1 stats
                # (j-outer; each chunk triggers its AllReduce, then its LN2
                # affine params are computed from the LOCAL stats rows so the
                # h2T affines -- and the whole FFN -- never wait on the
                # collective)
                # h2T lives on the right-side SBUF stack so its lifetime
                # (phase 4 .. end) can overlap pools on the left stack
                # without nesting; alloc'd only now because phases 1-3
                # (wv/Vall/hT/p3) are the SBUF high-water.
                h2pool = tc.alloc_tile_pool(name="h2T", bufs=1, side="right")
                h2T.extend(h2pool.tile([P, T], bf16, tag=f"h2{k}",
                                       name=f"h2_{k}") for k in range(CT))
                stp_bn1 = stat_tiles("bn1")
                bn1_io = [None, None]
                with tc.tile_pool(name="p4", bufs=1) as p4:
                    x2_sb = []
                    for k in range(CT):
                        x2k = p4.tile([P, T], fp32, tag=f"x2{k}",
                                      name=f"x2_{k}")
                        nc.sync.dma_start(x2k[:], xT_d[ts(k, P), :])
                        x2_sb.append(x2k)
                        u1.append(u1pool.tile([P, T], fp32, tag=f"u{k}",
                                              name=f"u1_{k}"))
                    # ---- fused LN2 params from LOCAL stats ----
                    # LN2(BN1(u1)) == u1*A + B with A = s/sqrt(s^2*v_c+eps),
                    # B = -mean_c(u1)*A, where v_c/mean_c are the per-(b,t)
                    # channel stats (exact, local) and s is the BN1 scale.
                    # s enters only through eps/s^2 ~= eps*v_c/g^2, so
                    # A ~= rsqrt(v_c*(1+eps) + eps^2) to ~1e-5 relative --
                    # no collective (and no g) needed.
                    ln2_rows_t = [None, None]

                    def ln2_rows(j):
                        """LN2 affine A/B rows for chunk j from local stats
                        (cheap single-lane chain; one ACT hop)."""
                        loc_j = bn1_io[j][0]
                        mc = h2pool.tile([1, CH], fp32, tag="l2m", bufs=1,
                                         name=f"l2m_{j}")
                        nc.vector.tensor_scalar_mul(mc[:], loc_j[:, 0:CH],
                                                    1.0 / C)
                        vc = h2pool.tile([1, CH], fp32, tag="l2v", bufs=1,
                                         name=f"l2v_{j}")
                        nc.vector.tensor_scalar_mul(vc[:],
                                                    loc_j[:, CH:2 * CH],
                                                    1.0 / C)
                        t1 = h2pool.tile([1, CH], fp32, tag="l2t1", bufs=1,
                                         name=f"l2t1_{j}")
                        t2 = h2pool.tile([1, CH], fp32, tag="l2t2", bufs=1,
                                         name=f"l2t2_{j}")
                        nc.vector.tensor_mul(t1[:], mc[:], mc[:])
                        nc.vector.tensor_sub(vc[:], vc[:], t1[:])
                        # t2 = A = 1/sqrt(v_c*(1+eps) + eps^2)
                        nc.vector.tensor_scalar_mul(t1[:], vc[:], 1.0 + EPS)
                        nc.vector.tensor_scalar_add(t1[:], t1[:], EPS * EPS)
                        nc.scalar.sqrt(t1[:], t1[:])
                        nc.vector.reciprocal_approx_fast(t2[:], t1[:])
                        # t1 = B = -mean_c * A
                        nc.vector.tensor_mul(t1[:], mc[:], t2[:])
                        nc.vector.tensor_scalar_mul(t1[:], t1[:], -1.0)
                        ln2_rows_t[j] = (t2, t1)

                    def ln2_apply(j):
                        """Broadcast A/B and write h2T chunk j = LN2(BN1(u1))
                        = u1*A + B."""
                        sl = slice(j * CH, (j + 1) * CH)
                        arow, brow = ln2_rows_t[j]
                        A = h2pool.tile([P, CH], fp32, tag="l2A", bufs=1,
                                        name=f"l2A_{j}")
                        bc_mm_into(A[:], arow[:], f"l2A_{j}")
                        Bt = h2pool.tile([P, CH], fp32, tag="l2B", bufs=1,
                                         name=f"l2B_{j}")
                        bc_mm_into(Bt[:], brow[:], f"l2B_{j}")
                        for k in range(CT):
                            affine_chunk(h2T[k][:, sl], u1[k][:, sl],
                                         A[:], Bt[:])

                    for j in range(NCH):
                        sl = slice(j * CH, (j + 1) * CH)
                        for k in range(CT):
                            wok = p4.tile([P, H, P], bf16, tag="wok",
                                          bufs=3, name=f"wok_{j}_{k}")
                            nc.sync.dma_start(wok[:], wo_d[k])
                            saps = ppw.tile([P, CH], fp32, tag="w",
                                            name=f"sa_ps_{k}_{j}")
                            for hh in range(H):
                                nc.tensor.matmul(saps[:], wok[:, hh, :],
                                                 o_nrm[hh][:, sl],
                                                 start=(hh == 0),
                                                 stop=(hh == H - 1))
                            nc.vector.scalar_tensor_tensor(
                                out=u1[k][:, sl], in0=saps[:],
                                scalar=bo_sb[:, k:k + 1], in1=x2_sb[k][:, sl],
                                op0=OP.add, op1=OP.add)
                            stats_chunk(u1[k][:, sl], stp_bn1[j],
                                        k == 0, k == CT - 1)
                            if j == 1 and k == 6:
                                # chunk-0 h2T: its A/B rows are long ready;
                                # the bc matmuls slot into the out-proj
                                # stream and the affines ride DVE slack.
                                ln2_apply(0)
                        bn1_io[j] = allreduce_chunk(u1pool, stp_bn1[j],
                                                    f"bn1_{j}")
                        ln2_rows(j)

              # ======= Phases 5-7: BN1 affine + FFN + BN2, sub-chunked ======
              # For each token sub-chunk: mm1 (h2T already affined), then the
              # BN1 global affine of u1 / previous sub-chunk's BN2 params ride
              # under mm2; mm2 + residual + BN2 stats; AllReduce; previous
              # sub-chunk's finale rides under this mm2.
              NS = len(FFN_SUB)
              stp_bn2 = stat_tiles("bn2", NS)
              bn2_io = [None] * NS
              with tc.tile_pool(name="p6", bufs=1) as p6, \
                   tc.tile_pool(name="pT", bufs=1) as pT:
                    def bn1_globals(j):
                        """Global BN1 affine params for 512-chunk j; applies
                        u1 <- BN1(u1) in place (gates only mm2's residual
                        add; the AllReduce has been in flight since phase 4)."""
                        sl = slice(j * CH, (j + 1) * CH)
                        g1r = pT.tile([1, CH], fp32, tag="rg", bufs=1,
                                      name=f"bn1g_{j}")
                        nc.sync.dma_start(g1r[:], bn1g_d[0:1, sl])
                        b1r = pT.tile([1, CH], fp32, tag="rgb", bufs=1,
                                      name=f"bn1b_{j}")
                        nc.sync.dma_start(b1r[:], bn1b_d[0:1, sl])
                        glob_j = bn1_io[j][1]
                        sc, bi = norm_params_chunk(
                            pT, glob_j[:, 0:CH], glob_j[:, CH:2 * CH],
                            NBC, f"bn1_{j}", g_row_sl=g1r[:], b_row_sl=b1r[:],
                            sc_tag="bnsc", bi_tag="bnbi", bc_bufs=1)
                        for k in range(CT):
                            affine_chunk(u1[k][:, sl], u1[k][:, sl],
                                         sc[:], bi[:])

                    def bn2_params(i, last=False):
                        """BN2 normalize params for sub-chunk i."""
                        off, width = FFN_SUB[i]
                        slw = slice(off, off + width)
                        g2r = pT.tile([1, CH], fp32, tag="rg", bufs=1,
                                      name=f"bn2g_{i}")
                        nc.sync.dma_start(g2r[:, :width], bn2g_d[0:1, slw])
                        b2r = pT.tile([1, CH], fp32, tag="rgb", bufs=1,
                                      name=f"bn2b_{i}")
                        nc.sync.dma_start(b2r[:, :width], bn2b_d[0:1, slw])
                        glob_i = bn2_io[i][1]
                        return norm_params_chunk(
                            pT, glob_i[:, 0:width], glob_i[:, width:2 * width],
                            NBC, f"bn2_{i}", g_row_sl=g2r[:, :width],
                            b_row_sl=b2r[:, :width], sc_tag="nsc",
                            bi_tag="nbi", bc_bufs=1, via_dma=not last,
                            width=width)

                    def bn2_finale_k(i, k, sc2, bi2):
                        off, width = FFN_SUB[i]
                        slw = slice(off, off + width)
                        yk = pT.tile([P, CH], fp32, tag="yout", bufs=2,
                                     name=f"y_{k}_{i}")
                        nc.vector.tensor_mul(yk[:, :width], u1[k][:, slw],
                                             sc2[:, :width])
                        nc.vector.tensor_add(yk[:, :width], yk[:, :width],
                                             bi2[:, :width])
                        nc.sync.dma_start(yT_d[ts(k, P), slw], yk[:, :width])

                    pp = None
                    for i, (off, width) in enumerate(FFN_SUB):
                        slw = slice(off, off + width)
                        # ---- FFN mm1 ----
                        z = []
                        for f in range(FT):
                            w1f = wstream.tile([P, CT, P], bf16, tag="w1f",
                                               bufs=2, name=f"w1f_{i}_{f}")
                            nc.sync.dma_start(w1f[:], w1_d[f])
                            zps = ppw.tile([P, CH], fp32, tag="w",
                                           name=f"z_ps_{i}_{f}")
                            for k in range(CT):
                                nc.tensor.matmul(zps[:, :width], w1f[:, k, :],
                                                 h2T[k][:, slw],
                                                 start=(k == 0),
                                                 stop=(k == CT - 1))
                            zf = p6.tile([P, CH], bf16, tag=f"z{f}",
                                         name=f"z_{i}_{f}")
                            nc.scalar.activation(zf[:, :width], zps[:, :width],
                                                 AF.Relu,
                                                 bias=b1_sb[:, f:f + 1],
                                                 scale=1.0)
                            z.append(zf)
                        # params consumed by this / the previous sub-chunk's
                        # mm2-side work; their collectives are long done
                        if i == 0:
                            ln2_apply(1)
                            bn1_globals(0)
                        elif i == 1:
                            bn1_globals(1)
                        if i > 0:
                            pp = bn2_params(i - 1)
                        # ---- FFN mm2 + residual + BN2 stats ----
                        for k in range(CT):
                            w2k = p6.tile([P, FT, P], bf16, tag="w2k", bufs=2,
                                          name=f"w2k_{i}_{k}")
                            nc.sync.dma_start(w2k[:], w2_d[k])
                            yps = ppw.tile([P, CH], fp32, tag="w",
                                           name=f"y_ps_{i}_{k}")
                            for f in range(FT):
                                nc.tensor.matmul(yps[:, :width], w2k[:, f, :],
                                                 z[f][:, :width],
                                                 start=(f == 0),
                                                 stop=(f == FT - 1))
                            nc.vector.scalar_tensor_tensor(
                                out=u1[k][:, slw], in0=yps[:, :width],
                                scalar=b2_sb[:, k:k + 1], in1=u1[k][:, slw],
                                op0=OP.add, op1=OP.add)
                            stats_chunk(u1[k][:, slw], stp_bn2[i],
                                        k == 0, k == CT - 1, width=width)
                            if i > 0:
                                bn2_finale_k(i - 1, k, pp[0], pp[1])
                        bn2_io[i] = allreduce_chunk(u1pool, stp_bn2[i],
                                                    f"bn2_{i}", width=width)
                    # last sub-chunk: PE is idle, use the low-latency matmul
                    # broadcast instead of the DMA round trip
                    pp = bn2_params(NS - 1, last=True)
                    for k in range(CT):
                        bn2_finale_k(NS - 1, k, pp[0], pp[1])
              h2pool.release()

    nc.compile()
    return nc


def _get_program():
    global _PROG
    if _PROG is None:
        _PROG = _build()
    return _PROG


def _prep_shared(inputs):
    """Host-side weight folding + pre-tiling; identical for every core."""
    f32 = np.float32
    bf16 = ml_dtypes.bfloat16
    wq = np.asarray(inputs["wq"], f32)      # (H, C, D)
    wk = np.asarray(inputs["wk"], f32)
    wv = np.asarray(inputs["wv"], f32)
    wo = np.asarray(inputs["wo"], f32)      # (C, C)
    bo = np.asarray(inputs["bo"], f32)      # (C,)
    g1 = np.asarray(inputs["ln1_g"], f32)
    b1n = np.asarray(inputs["ln1_b"], f32)
    g2 = np.asarray(inputs["ln2_g"], f32)
    b2n = np.asarray(inputs["ln2_b"], f32)
    w1 = np.asarray(inputs["w1"], f32)      # (C, F)
    b1 = np.asarray(inputs["b1"], f32)      # (F,)
    w2 = np.asarray(inputs["w2"], f32)      # (F, C)
    b2 = np.asarray(inputs["b2"], f32)      # (C,)

    dscale = f32(D) ** f32(-0.5)
    # fold ln1 affine into qkv projections; q also takes 1/sqrt(D)
    wq2 = (wq * g1[None, :, None] * dscale).transpose(1, 0, 2).reshape(C, C)
    wk2 = (wk * g1[None, :, None]).transpose(1, 0, 2).reshape(C, C)
    wv2 = (wv * g1[None, :, None]).transpose(1, 0, 2).reshape(C, C)
    bq = (np.einsum("c,hcd->hd", b1n, wq) * dscale).reshape(C)
    bv = np.einsum("c,hcd->hd", b1n, wv).reshape(C)
    # k-side bias cancels in softmax (constant per row); v bias folds into bo
    bo2 = bo + bv @ wo
    w1f = g2[:, None] * w1
    b1f = b1 + b2n @ w1

    def lhst_tiles(w, n_out):
        # (C_in, n_out*P) -> (n_out, P, C_in//P, P):
        # [o, p, ki, n] = w[ki*P + p, o*P + n]
        ci = w.shape[0]
        return np.ascontiguousarray(
            w.reshape(ci // P, P, n_out, P).transpose(2, 1, 0, 3)
        ).astype(bf16)

    def cols(v, n):  # (n*P,) -> (P, n) with [p, i] = v[i*P + p]
        return np.ascontiguousarray(v.reshape(n, P).T, dtype=f32)

    def row(v):
        return np.ascontiguousarray(v.reshape(1, T), dtype=f32)

    return dict(
        wq=lhst_tiles(wq2, H), wk=lhst_tiles(wk2, H),
        wv=wv2.astype(bf16),
        bq=cols(bq, H), wo=lhst_tiles(wo, CT), bo=cols(bo2, CT),
        w1=lhst_tiles(w1f, FT), b1=cols(b1f, FT),
        w2=lhst_tiles(w2, CT), b2=cols(b2, CT),
        bn1g=row(np.asarray(inputs["bn1_g"], f32)),
        bn1b=row(np.asarray(inputs["bn1_b"], f32)),
        bn2g=row(np.asarray(inputs["bn2_g"], f32)),
        bn2b=row(np.asarray(inputs["bn2_b"], f32)),
    )


def _run(inputs, trace=False):
    from concourse import bass_utils
    nc = _get_program()
    x = np.asarray(inputs["x"], np.float32)
    shared = _prep_shared(inputs)
    in_maps = []
    for b in range(B):
        m = dict(shared)
        xt = np.ascontiguousarray(x[b].T)
        m["xT"] = xt
        m["xbf"] = xt.astype(ml_dtypes.bfloat16)
        in_maps.append(m)
    res = bass_utils.run_bass_kernel_spmd(
        nc, in_maps, core_ids=list(range(NCORES)), trace=trace)
    out = np.stack([res.results[b]["yT"].T for b in range(B)]).astype(np.float32)
    return out, res


def kernel(**inputs):
    out, _ = _run(inputs, trace=False)
    return out


# revision 15
# speedup vs baseline: 1.0322x; 1.0322x over previous
"""Trainium2 Bass kernel for nn_Block_50113678410401 (dense transformer block).

Strategy: data-parallel over the batch axis (B=8 -> 8 NeuronCores, one batch
element per core). All on-chip activations live in "layout A": feature axis on
SBUF partitions, token axis (T) on the free dimension, so no on-chip
transposes are needed (host pre-transposes x and post-transposes the output).

Per core:
  LN1 (stats via ones-matmul over partitions), per-head causal attention
  (no-max-sub exp softmax, denominator via ones-matmul, normalization via
  K=1 broadcast matmul + wide reciprocal), output projection + residual,
  BatchNorm over (B,C) with per-chunk cross-core AllReduces of (sum, sumsq)
  per T channel, LN2, FFN (C -> 4C -> relu -> C), residual, second BatchNorm.

Collective-latency hiding:
  * a tiny warm-up AllReduce at kernel start absorbs the first-collective
    ramp cost (~50us observed on the first op of a NEFF).
  * the LN2 affine of BN1(u1) is computed from LOCAL per-core stats only
    (the global BN1 scale enters the exact formula only through an eps/s^2
    term; substituting the local-sample estimate of s changes the result by
    ~1e-7 relative).  So the FFN starts immediately after the out-projection
    without waiting for the BN1 AllReduce; the global BN1 params are only
    needed for the residual affine consumed by mm2's residual-add, ~150us
    after the collective is triggered.
  * the FFN runs in token sub-chunks of 512/256/256 columns; each sub-chunk
    triggers its BN2 stats AllReduce when its mm2 finishes, so the final
    (tail) AllReduce covers only 256 columns and its finale is short.

All big matmuls run in bf16 with fp32 PSUM accumulation; statistics,
softmax, residuals and normalizations are fp32. Weights arrive host-pretiled
so every weight DMA is contiguous per partition.

LayerNorm/projection affine parameters are folded into the weights on the
host: wq' = diag(ln1_g) wq / sqrt(D) (q also carries 1/sqrt(D)), k-side bias
drops out of softmax by shift invariance, v-side bias is folded into the
output-projection bias, ln2 affine is folded into w1/b1.
"""

import numpy as np
import ml_dtypes

B, T, C, H, D = 8, 1024, 1536, 12, 128
F = 4 * C            # 6144
P = 128
CT = C // P          # 12 c-tiles
FT = F // P          # 48 f-tiles
ST = T // P          # 8 s-tiles
CH = 512             # matmul free-dim chunk
NCH = T // CH        # 2 chunks
EPS = 1e-5
NCORES = 8
NBC = B * C          # BatchNorm count over (B, C)

# FFN token sub-chunks (offset, width).  512-wide: the w1/w2 weight stream
# (18.9 MB per pass) needs the full pass duration to fit in HBM bandwidth;
# narrower chunks starve the PE on weight DMA.
FFN_SUB = [(0, 512), (512, 512)]

_PROG = None


def _build():
    import concourse.bass as bass
    import concourse.mybir as mybir
    import concourse.tile as tile
    from concourse import bacc
    from concourse.masks import make_upper_triangular

    fp32 = mybir.dt.float32
    bf16 = mybir.dt.bfloat16
    AF = mybir.ActivationFunctionType
    OP = mybir.AluOpType
    ts = bass.ts

    nc = bacc.Bacc("TRN2", target_bir_lowering=False, debug=False,
                   enable_asserts=True, num_devices=NCORES)

    # ---- DRAM I/O (weights host-pretiled for contiguous DMA) ----
    xT_d = nc.dram_tensor("xT", (C, T), fp32, kind="ExternalInput").ap()
    xbf_d = nc.dram_tensor("xbf", (C, T), bf16, kind="ExternalInput").ap()
    wq_d = nc.dram_tensor("wq", (H, P, CT, P), bf16, kind="ExternalInput").ap()
    wk_d = nc.dram_tensor("wk", (H, P, CT, P), bf16, kind="ExternalInput").ap()
    wv_d = nc.dram_tensor("wv", (C, C), bf16, kind="ExternalInput").ap()
    bq_d = nc.dram_tensor("bq", (P, H), fp32, kind="ExternalInput").ap()
    wo_d = nc.dram_tensor("wo", (CT, P, H, P), bf16, kind="ExternalInput").ap()
    bo_d = nc.dram_tensor("bo", (P, CT), fp32, kind="ExternalInput").ap()
    w1_d = nc.dram_tensor("w1", (FT, P, CT, P), bf16, kind="ExternalInput").ap()
    b1_d = nc.dram_tensor("b1", (P, FT), fp32, kind="ExternalInput").ap()
    w2_d = nc.dram_tensor("w2", (CT, P, FT, P), bf16, kind="ExternalInput").ap()
    b2_d = nc.dram_tensor("b2", (P, CT), fp32, kind="ExternalInput").ap()
    bn1g_d = nc.dram_tensor("bn1g", (1, T), fp32, kind="ExternalInput").ap()
    bn1b_d = nc.dram_tensor("bn1b", (1, T), fp32, kind="ExternalInput").ap()
    bn2g_d = nc.dram_tensor("bn2g", (1, T), fp32, kind="ExternalInput").ap()
    bn2b_d = nc.dram_tensor("bn2b", (1, T), fp32, kind="ExternalInput").ap()
    yT_d = nc.dram_tensor("yT", (C, T), fp32, kind="ExternalOutput").ap()

    with tile.TileContext(nc) as tc:
        with tc.tile_pool(name="const", bufs=1) as cpool, \
             tc.tile_pool(name="scratch", bufs=1) as spool, \
             tc.tile_pool(name="u1p", bufs=1) as u1pool, \
             tc.tile_pool(name="wstr", bufs=1) as wstream, \
             tc.tile_pool(name="ppw", bufs=6, space="PSUM") as ppw, \
             tc.tile_pool(name="pps", bufs=2, space="PSUM") as pps, \
             tc.tile_pool(name="dram", bufs=1, space="DRAM") as dpool:

            # ---- constants ----
            ones_bf = cpool.tile([P, 1], bf16, name="ones_bf")
            nc.vector.memset(ones_bf[:], 1.0)
            trimask = cpool.tile([P, P], bf16, name="trimask")
            make_upper_triangular(nc, trimask[:], val=1.0, diag=True)
            bq_sb = cpool.tile([P, H], fp32, name="bq_sb")
            nc.sync.dma_start(bq_sb[:], bq_d[:])
            bo_sb = cpool.tile([P, CT], fp32, name="bo_sb")
            nc.sync.dma_start(bo_sb[:], bo_d[:])
            b1_sb = cpool.tile([P, FT], fp32, name="b1_sb")
            nc.sync.dma_start(b1_sb[:], b1_d[:])
            b2_sb = cpool.tile([P, CT], fp32, name="b2_sb")
            nc.sync.dma_start(b2_sb[:], b2_d[:])

            # Warm-up AllReduce: the first collective of a NEFF pays a large
            # ramp cost (~50us observed); burn it on a 64B dummy during the
            # attention phase so the real BN AllReduces run at steady-state
            # latency.
            wrow = cpool.tile([1, 16], fp32, name="warm_cc_row")
            nc.vector.memset(wrow[:], 0.0)
            wcin = dpool.tile([1, 16], fp32, name="warm_cc_in")
            wcout = dpool.tile([1, 16], fp32, name="warm_cc_out")
            nc.gpsimd.dma_start(wcin[:], wrow[:])
            nc.gpsimd.collective_compute(
                "AllReduce", mybir.AluOpType.add,
                replica_groups=[list(range(NCORES))],
                ins=[wcin.opt()], outs=[wcout.opt()],
            )

            # PE clock warm-up: ~5us of junk matmuls during the initial
            # x DMA so the LN1 stats matmuls run at full clock.
            warm_ps = ppw.tile([P, P], fp32, tag="w", name="warm_ps")
            for _ in range(40):
                nc.tensor.matmul(warm_ps[:], trimask[:], trimask[:],
                                 start=True, stop=True)

            # ---- helpers ----
            def bcast_into(dst_ap, row_ap, name, n=CH, eng=None):
                """(1, n) fp32 SBUF row -> (P, n) via DRAM bounce on the given
                DMA-capable engine queue (default scalar)."""
                e = eng if eng is not None else nc.scalar
                dr = dpool.tile([1, n], fp32, name=f"{name}_dr")
                e.dma_start(dr[:], row_ap)
                e.dma_start(dst_ap, dr[:].to_broadcast((P, n)))

            ones1f = cpool.tile([1, P], fp32, name="ones1f")
            nc.vector.memset(ones1f[:], 1.0)

            def bc_mm_into(dst_ap, row_ap, name, n=CH):
                """(1, n) fp32 row -> (P, n) SBUF via K=1 matmul + copy.
                Higher PE cost than bcast_into but ~3x lower latency; used on
                norm-param critical paths where the PE is idle anyway."""
                ps = ppw.tile([P, CH], fp32, tag="w", name=f"{name}_ps")
                nc.tensor.matmul(ps[:, :n], ones1f[:], row_ap, start=True,
                                 stop=True)
                nc.scalar.copy(dst_ap, ps[:, :n])

            # Packed stat psum tile: row 0 accumulates sum, row 32 sumsq.
            def stat_tiles(name, cnt=NCH):
                return [pps.tile([P, CH], fp32, tag="st", bufs=2,
                                 name=f"{name}_{j}") for j in range(cnt)]

            def stats_chunk(src_ap, stp_j, first, last, is_bf16=False,
                            width=CH):
                """Ones-matmul partial sums of src chunk ((P,width)) and its
                square into packed stat rows."""
                if is_bf16:
                    cbf = src_ap
                else:
                    cbf_t = spool.tile([P, CH], bf16, tag="cast_bf", bufs=2,
                                       name="cbf")
                    nc.vector.tensor_copy(cbf_t[:, :width], src_ap)
                    cbf = cbf_t[:, :width]
                csq = spool.tile([P, CH], bf16, tag="cast_sq", bufs=2,
                                 name="csq")
                nc.scalar.square(csq[:, :width], src_ap)
                nc.tensor.matmul(stp_j[0:1, :width], ones_bf[:], cbf,
                                 start=first, stop=last)
                nc.tensor.matmul(stp_j[32:33, :width], ones_bf[:],
                                 csq[:, :width], start=first, stop=last)

            def allreduce_chunk(pool, stp_j, name, width=CH):
                """AllReduce-add this chunk's packed (sum, sumsq) across
                cores. Returns the (1, 2*width) (local, global) rows."""
                loc = pool.tile([1, 2 * CH], fp32, tag="arloc", bufs=1,
                                name=f"{name}_loc")
                nc.scalar.copy(loc[:, 0:width], stp_j[0:1, :width])
                nc.scalar.copy(loc[:, width:2 * width], stp_j[32:33, :width])
                cin = dpool.tile([1, 2 * width], fp32, name=f"{name}_cin")
                cout = dpool.tile([1, 2 * width], fp32, name=f"{name}_cout")
                nc.gpsimd.dma_start(cin[:], loc[:, :2 * width])
                nc.gpsimd.collective_compute(
                    "AllReduce", mybir.AluOpType.add,
                    replica_groups=[list(range(NCORES))],
                    ins=[cin.opt()], outs=[cout.opt()],
                )
                glob = pool.tile([1, 2 * CH], fp32, tag="arglob", bufs=1,
                                 name=f"{name}_glob")
                nc.gpsimd.dma_start(glob[:, :2 * width], cout[:])
                return loc, glob

            def norm_params_chunk(pool, s1_ap, s2_ap, count, name,
                                  g_row_sl=None, b_row_sl=None,
                                  sc_tag="nsc", bi_tag="nbi", bc_pool=None,
                                  bc_bufs=1, via_dma=False, width=CH):
                """Per-chunk normalization params, computed at row level
                (single-lane, cheap custom-DVE reciprocal), then broadcast
                to (P, width). Returns (sc_bc, bi_bc)."""
                m = pool.tile([1, CH], fp32, tag="rm", bufs=1,
                              name=f"{name}_m")
                nc.vector.tensor_scalar_mul(m[:, :width], s1_ap, 1.0 / count)
                v = pool.tile([1, CH], fp32, tag="rv", bufs=1,
                              name=f"{name}_v")
                nc.vector.tensor_scalar_mul(v[:, :width], s2_ap, 1.0 / count)
                bias = pool.tile([1, CH], fp32, tag="rb", bufs=1,
                                 name=f"{name}_brow")
                nc.vector.tensor_mul(bias[:, :width], m[:, :width],
                                     m[:, :width])
                nc.vector.tensor_sub(v[:, :width], v[:, :width],
                                     bias[:, :width])
                nc.vector.tensor_scalar_add(v[:, :width], v[:, :width], EPS)
                nc.scalar.sqrt(v[:, :width], v[:, :width])
                scale = pool.tile([1, CH], fp32, tag="rs", bufs=1,
                                  name=f"{name}_srow")
                if g_row_sl is not None:
                    rc = pool.tile([1, CH], fp32, tag="rr", bufs=1,
                                   name=f"{name}_rc")
                    nc.vector.reciprocal_approx_fast(rc[:, :width],
                                                     v[:, :width])
                    nc.vector.tensor_mul(scale[:, :width], rc[:, :width],
                                         g_row_sl)
                else:
                    nc.vector.reciprocal_approx_fast(scale[:, :width],
                                                     v[:, :width])
                nc.vector.tensor_mul(bias[:, :width], m[:, :width],
                                     scale[:, :width])
                nc.vector.tensor_scalar_mul(bias[:, :width], bias[:, :width],
                                            -1.0)
                if b_row_sl is not None:
                    nc.vector.tensor_add(bias[:, :width], bias[:, :width],
                                         b_row_sl)
                bpool = bc_pool if bc_pool is not None else pool
                sc_bc = bpool.tile([P, CH], fp32, tag=sc_tag, bufs=bc_bufs,
                                   name=f"{name}_scbc")
                bi_bc = bpool.tile([P, CH], fp32, tag=bi_tag, bufs=bc_bufs,
                                   name=f"{name}_bibc")
                if via_dma:
                    bcast_into(sc_bc[:, :width], scale[:, :width],
                               f"{name}_sc", n=width, eng=nc.gpsimd)
                    bcast_into(bi_bc[:, :width], bias[:, :width],
                               f"{name}_bi", n=width, eng=nc.gpsimd)
                else:
                    bc_mm_into(sc_bc[:, :width], scale[:, :width],
                               f"{name}_sc", n=width)
                    bc_mm_into(bi_bc[:, :width], bias[:, :width],
                               f"{name}_bi", n=width)
                return sc_bc, bi_bc

            def affine_chunk(dst_ap, src_ap, sc_ap, bi_ap, width=CH):
                """dst = src * sc + bi on one (P, width) chunk."""
                tmp = spool.tile([P, CH], fp32, tag="ntmp", bufs=2,
                                 name="ntmp")
                nc.vector.tensor_mul(tmp[:, :width], src_ap, sc_ap)
                nc.vector.tensor_add(dst_ap, tmp[:, :width], bi_ap)

            u1 = []     # created at phase 4 (first use)
            o_nrm = []  # created at phase 3
            h2pool = None  # right-side pool, alloc'd after phase 1's peak
            h2T = []

            if True:
              with tc.tile_pool(name="onrm", bufs=1) as opool:
                with tc.tile_pool(name="hT", bufs=1) as hpool:
                    # hT split per column chunk (separate tiles) so chunk-0
                    # stats/params/affines/V start after only half the x DMA
                    # (dependency tracking is tile-granular).
                    hT = [[hpool.tile([P, CH], bf16, tag=f"h{k}_{j}",
                                      name=f"hT_{k}_{j}")
                           for j in range(NCH)] for k in range(CT)]

                    def hT_s(k, s):
                        """(P, P) slice of hT for s-tile s."""
                        j = (s * P) // CH
                        return hT[k][j][:, s * P - j * CH:(s + 1) * P - j * CH]

                    # ================= Phase 1: LN1 =================
                    with tc.tile_pool(name="vall2", bufs=1) as vpool:
                        Vall = [vpool.tile([P, C], bf16, tag=f"v{s}",
                                           name=f"V_{s}") for s in range(ST)]
                        with tc.tile_pool(name="p1", bufs=1) as p1:
                            stp = stat_tiles("ln1")
                            for j in range(NCH):
                                sl = slice(j * CH, (j + 1) * CH)
                                for k in range(CT):
                                    nc.sync.dma_start(hT[k][j][:],
                                                      xbf_d[ts(k, P), sl])
                                for k in range(CT):
                                    stats_chunk(hT[k][j][:], stp[j], k == 0,
                                                k == CT - 1, is_bf16=True)
                            ln1p = []
                            for j in range(NCH):
                                ln1p.append(norm_params_chunk(
                                    p1, stp[j][0:1, :], stp[j][32:33, :], C,
                                    f"ln1_{j}", bc_bufs=2))
                            # chunk-0 affines, V for the first four s-tiles,
                            # then chunk-1 affines, V for the rest.
                            sc0, bi0 = ln1p[0][0], ln1p[0][1]
                            for k in range(CT):
                                affine_chunk(hT[k][0][:], hT[k][0][:],
                                             sc0[:], bi0[:])

                            with tc.tile_pool(name="wv", bufs=1) as wvpool:
                                def v_pass(s_lo, s_hi, tag):
                                    for n in range(C // CH):
                                        wv_sb = []
                                        for k in range(CT):
                                            wvk = wvpool.tile(
                                                [P, CH], bf16, tag=f"wv{k}",
                                                bufs=2,
                                                name=f"wv_{k}_{n}_{tag}")
                                            nc.sync.dma_start(
                                                wvk[:],
                                                wv_d[ts(k, P), ts(n, CH)])
                                            wv_sb.append(wvk)
                                        for s in range(s_lo, s_hi):
                                            vps = ppw.tile(
                                                [P, CH], fp32, tag="w",
                                                name=f"v_ps_{s}_{n}")
                                            for k in range(CT):
                                                nc.tensor.matmul(
                                                    vps[:], hT_s(k, s),
                                                    wv_sb[k][:],
                                                    start=(k == 0),
                                                    stop=(k == CT - 1))
                                            nc.scalar.copy(
                                                Vall[s][:, ts(n, CH)], vps[:])

                                v_pass(0, ST // 2, "a")
                                sc1, bi1 = ln1p[1][0], ln1p[1][1]
                                for k in range(CT):
                                    affine_chunk(hT[k][1][:], hT[k][1][:],
                                                 sc1[:], bi1[:])
                                v_pass(ST // 2, ST, "b")

                        # ============ Phase 3: per-head attention ==========
                        with tc.tile_pool(name="p3", bufs=1) as p3:
                            for h in range(H):
                                o_nrm.append(opool.tile(
                                    [P, T], bf16, tag=f"o{h}", name=f"on_{h}"))

                            def qk_proj(h):
                                """Q/K projection matmuls + PSUM->SBUF
                                copies for head h."""
                                wqh = p3.tile([P, CT, P], bf16, tag="wqh",
                                              bufs=2, name=f"wqh_{h}")
                                nc.sync.dma_start(wqh[:], wq_d[h])
                                wkh = p3.tile([P, CT, P], bf16, tag="wkh",
                                              bufs=2, name=f"wkh_{h}")
                                nc.sync.dma_start(wkh[:], wk_d[h])
                                qT = p3.tile([P, T], bf16, tag="qT", bufs=2,
                                             name=f"qT_{h}")
                                kT = p3.tile([P, T], bf16, tag="kT", bufs=2,
                                             name=f"kT_{h}")
                                # k-outer, j-inner: consecutive matmuls
                                # share the same stationary weight tile
                                qps = [ppw.tile([P, CH], fp32, tag="w",
                                                name=f"q_ps_{h}_{j}")
                                       for j in range(NCH)]
                                kps = [ppw.tile([P, CH], fp32, tag="w",
                                                name=f"k_ps_{h}_{j}")
                                       for j in range(NCH)]
                                for k in range(CT):
                                    for j in range(NCH):
                                        nc.tensor.matmul(qps[j][:],
                                                         wqh[:, k, :],
                                                         hT[k][j][:],
                                                         start=(k == 0),
                                                         stop=(k == CT - 1))
                                for k in range(CT):
                                    for j in range(NCH):
                                        nc.tensor.matmul(kps[j][:],
                                                         wkh[:, k, :],
                                                         hT[k][j][:],
                                                         start=(k == 0),
                                                         stop=(k == CT - 1))
                                for j in range(NCH):
                                    sl = slice(j * CH, (j + 1) * CH)
                                    nc.scalar.activation(qT[:, sl], qps[j][:],
                                                         AF.Identity,
                                                         bias=bq_sb[:, h:h + 1],
                                                         scale=1.0)
                                    nc.scalar.copy(kT[:, sl], kps[j][:])
                                return qT, kT

                            def scores_exp(h, qT, kT):
                                """Causal scores + exp for head h (s-tile
                                covers t >= s*P)."""
                                aT = []
                                for s in range(ST):
                                    at = p3.tile([P, T - s * P], bf16,
                                                 tag=f"a{s}", bufs=1,
                                                 name=f"aT_{h}_{s}")
                                    aT.append(at)
                                    for j in range(NCH):
                                        lo = max(j * CH, s * P)
                                        hi = (j + 1) * CH
                                        if lo >= hi:
                                            continue
                                        sps = ppw.tile([P, CH], fp32, tag="w",
                                                       name=f"s_ps_{h}_{s}_{j}")
                                        nc.tensor.matmul(sps[:, :hi - lo],
                                                         kT[:, ts(s, P)],
                                                         qT[:, lo:hi],
                                                         start=True, stop=True)
                                        nc.scalar.activation(
                                            at[:, lo - s * P:hi - s * P],
                                            sps[:, :hi - lo], AF.Exp)
                                    nc.vector.tensor_mul(at[:, 0:P],
                                                         at[:, 0:P],
                                                         trimask[:])
                                return aT

                            def den_av(h, aT):
                                """Denominators, 1/den broadcast, attn @ V,
                                normalize into o_nrm[h]."""
                                den_ps = pps.tile([P, CH], fp32, tag="st",
                                                  bufs=2, name=f"dn_{h}")
                                for j in range(NCH):
                                    r0 = 32 * j
                                    smax = min(ST, 4 * (j + 1))
                                    for s in range(smax):
                                        lo = max(0, s * P - j * CH)
                                        nc.tensor.matmul(
                                            den_ps[r0:r0 + 1, lo:CH],
                                            ones_bf[:],
                                            aT[s][:, j * CH + lo - s * P:
                                                  (j + 1) * CH - s * P],
                                            start=(s == 0), stop=(s == smax - 1))
                                # 1/den rows right off the den psum; the
                                # (P, CH) broadcast rides the PE (K=1 matmul,
                                # deterministic) instead of a DRAM DMA bounce
                                # -- DMA latency variance here accumulates as
                                # cross-core skew that the final AllReduce
                                # pays for.
                                rjs = []
                                for j in range(NCH):
                                    dj = p3.tile([1, CH], fp32, tag="den",
                                                 bufs=2, name=f"den_{h}_{j}")
                                    nc.scalar.copy(
                                        dj[:], den_ps[32 * j:32 * j + 1, :])
                                    rj = p3.tile([1, CH], fp32, tag="rrow",
                                                 bufs=2, name=f"rr_{h}_{j}")
                                    nc.vector.reciprocal_approx_fast(
                                        rj[:], dj[:])
                                    rjs.append(rj)
                                r_bc = p3.tile([P, T], fp32, tag="rbc", bufs=2,
                                               name=f"rbc_{h}")
                                opss = []
                                for j in range(NCH):
                                    smax = min(ST, 4 * (j + 1))
                                    ops_ = ppw.tile([P, CH], fp32, tag="w",
                                                    name=f"o_ps_{h}_{j}")
                                    opss.append(ops_)
                                    for s in range(smax):
                                        lo = max(0, s * P - j * CH)
                                        nc.tensor.matmul(
                                            ops_[:, lo:CH],
                                            Vall[s][:, ts(h, P)],
                                            aT[s][:, j * CH + lo - s * P:
                                                  (j + 1) * CH - s * P],
                                            start=(s == 0), stop=(s == smax - 1))
                                for j in range(NCH):
                                    sl = slice(j * CH, (j + 1) * CH)
                                    bc_mm_into(r_bc[:, sl], rjs[j][:],
                                               f"rbc_{h}_{j}")
                                    nc.vector.tensor_mul(o_nrm[h][:, sl],
                                                         opss[j][:],
                                                         r_bc[:, sl])

                            # Software pipeline: head h-1's denominator /
                            # attn@V matmuls are emitted AFTER head h's Q/K
                            # projection matmuls, so the PE never waits
                            # head-of-line on head h-1's exp (ACT) -- it does
                            # head h's projections meanwhile; den/attnV then
                            # dispatch with their inputs long ready.
                            prev = None
                            for h in range(H):
                                qkt = qk_proj(h)
                                if prev is not None:
                                    den_av(prev[0], prev[1])
                                aT = scores_exp(h, *qkt)
                                prev = (h, aT)
                            den_av(prev[0], prev[1])

                # hT closed; Phase 4: out-proj + residual + BN1 stats
                # (j-outer; each chunk triggers its AllReduce, then its LN2
                # affine params are computed from the LOCAL stats rows so the
                # h2T affines -- and the whole FFN -- never wait on the
                # collective)
                wcin2 = dpool.tile([1, 16], fp32, name="skew_cc_in")
                wcout2 = dpool.tile([1, 16], fp32, name="skew_cc_out")
                nc.gpsimd.dma_start(wcin2[:], wrow[:])
                nc.gpsimd.collective_compute(
                    "AllReduce", mybir.AluOpType.add,
                    replica_groups=[list(range(NCORES))],
                    ins=[wcin2.opt()], outs=[wcout2.opt()],
                )
                # h2T lives on the right-side SBUF stack so its lifetime
                # (phase 4 .. end) can overlap pools on the left stack
                # without nesting; alloc'd only now because phases 1-3
                # (wv/Vall/hT/p3) are the SBUF high-water.
                h2pool = tc.alloc_tile_pool(name="h2T", bufs=1, side="right")
                h2T.extend(h2pool.tile([P, T], bf16, tag=f"h2{k}",
                                       name=f"h2_{k}") for k in range(CT))
                stp_bn1 = stat_tiles("bn1")
                bn1_io = [None, None]
                with tc.tile_pool(name="p4", bufs=1) as p4:
                    x2_sb = []
                    for k in range(CT):
                        x2k = p4.tile([P, T], fp32, tag=f"x2{k}",
                                      name=f"x2_{k}")
                        nc.sync.dma_start(x2k[:], xT_d[ts(k, P), :])
                        x2_sb.append(x2k)
                        u1.append(u1pool.tile([P, T], fp32, tag=f"u{k}",
                                              name=f"u1_{k}"))
                    # ---- fused LN2 params from LOCAL stats ----
                    # LN2(BN1(u1)) == u1*A + B with A = s/sqrt(s^2*v_c+eps),
                    # B = -mean_c(u1)*A, where v_c/mean_c are the per-(b,t)
                    # channel stats (exact, local) and s is the BN1 scale.
                    # s enters only through eps/s^2 ~= eps*v_c/g^2, so
                    # A ~= rsqrt(v_c*(1+eps) + eps^2) to ~1e-5 relative --
                    # no collective (and no g) needed.
                    ln2_rows_t = [None, None]

                    def ln2_rows(j):
                        """LN2 affine A/B rows for chunk j from local stats
                        (cheap single-lane chain; one ACT hop)."""
                        loc_j = bn1_io[j][0]
                        mc = h2pool.tile([1, CH], fp32, tag="l2m", bufs=1,
                                         name=f"l2m_{j}")
                        nc.vector.tensor_scalar_mul(mc[:], loc_j[:, 0:CH],
                                                    1.0 / C)
                        vc = h2pool.tile([1, CH], fp32, tag="l2v", bufs=1,
                                         name=f"l2v_{j}")
                        nc.vector.tensor_scalar_mul(vc[:],
                                                    loc_j[:, CH:2 * CH],
                                                    1.0 / C)
                        t1 = h2pool.tile([1, CH], fp32, tag="l2t1", bufs=1,
                                         name=f"l2t1_{j}")
                        t2 = h2pool.tile([1, CH], fp32, tag="l2t2", bufs=1,
                                         name=f"l2t2_{j}")
                        nc.vector.tensor_mul(t1[:], mc[:], mc[:])
                        nc.vector.tensor_sub(vc[:], vc[:], t1[:])
                        # t2 = A = 1/sqrt(v_c*(1+eps) + eps^2)
                        nc.vector.tensor_scalar_mul(t1[:], vc[:], 1.0 + EPS)
                        nc.vector.tensor_scalar_add(t1[:], t1[:], EPS * EPS)
                        nc.scalar.sqrt(t1[:], t1[:])
                        nc.vector.reciprocal_approx_fast(t2[:], t1[:])
                        # t1 = B = -mean_c * A
                        nc.vector.tensor_mul(t1[:], mc[:], t2[:])
                        nc.vector.tensor_scalar_mul(t1[:], t1[:], -1.0)
                        ln2_rows_t[j] = (t2, t1)

                    def ln2_apply(j):
                        """Broadcast A/B and write h2T chunk j = LN2(BN1(u1))
                        = u1*A + B."""
                        sl = slice(j * CH, (j + 1) * CH)
                        arow, brow = ln2_rows_t[j]
                        A = h2pool.tile([P, CH], fp32, tag="l2A", bufs=1,
                                        name=f"l2A_{j}")
                        bc_mm_into(A[:], arow[:], f"l2A_{j}")
                        Bt = h2pool.tile([P, CH], fp32, tag="l2B", bufs=1,
                                         name=f"l2B_{j}")
                        bc_mm_into(Bt[:], brow[:], f"l2B_{j}")
                        for k in range(CT):
                            affine_chunk(h2T[k][:, sl], u1[k][:, sl],
                                         A[:], Bt[:])

                    for j in range(NCH):
                        sl = slice(j * CH, (j + 1) * CH)
                        for k in range(CT):
                            wok = p4.tile([P, H, P], bf16, tag="wok",
                                          bufs=3, name=f"wok_{j}_{k}")
                            nc.sync.dma_start(wok[:], wo_d[k])
                            saps = ppw.tile([P, CH], fp32, tag="w",
                                            name=f"sa_ps_{k}_{j}")
                            for hh in range(H):
                                nc.tensor.matmul(saps[:], wok[:, hh, :],
                                                 o_nrm[hh][:, sl],
                                                 start=(hh == 0),
                                                 stop=(hh == H - 1))
                            nc.vector.scalar_tensor_tensor(
                                out=u1[k][:, sl], in0=saps[:],
                                scalar=bo_sb[:, k:k + 1], in1=x2_sb[k][:, sl],
                                op0=OP.add, op1=OP.add)
                            stats_chunk(u1[k][:, sl], stp_bn1[j],
                                        k == 0, k == CT - 1)
                            if j == 1 and k == 6:
                                # chunk-0 h2T: its A/B rows are long ready;
                                # the bc matmuls slot into the out-proj
                                # stream and the affines ride DVE slack.
                                ln2_apply(0)
                        bn1_io[j] = allreduce_chunk(u1pool, stp_bn1[j],
                                                    f"bn1_{j}")
                        ln2_rows(j)

              # ======= Phases 5-7: BN1 affine + FFN + BN2, sub-chunked ======
              # For each token sub-chunk: mm1 (h2T already affined), then the
              # BN1 global affine of u1 / previous sub-chunk's BN2 params ride
              # under mm2; mm2 + residual + BN2 stats; AllReduce; previous
              # sub-chunk's finale rides under this mm2.
              NS = len(FFN_SUB)
              stp_bn2 = stat_tiles("bn2", NS)
              bn2_io = [None] * NS
              with tc.tile_pool(name="p6", bufs=1) as p6, \
                   tc.tile_pool(name="pT", bufs=1) as pT:
                    def bn1_globals(j):
                        """Global BN1 affine params for 512-chunk j; applies
                        u1 <- BN1(u1) in place (gates only mm2's residual
                        add; the AllReduce has been in flight since phase 4)."""
                        sl = slice(j * CH, (j + 1) * CH)
                        g1r = pT.tile([1, CH], fp32, tag="rg", bufs=1,
                                      name=f"bn1g_{j}")
                        nc.sync.dma_start(g1r[:], bn1g_d[0:1, sl])
                        b1r = pT.tile([1, CH], fp32, tag="rgb", bufs=1,
                                      name=f"bn1b_{j}")
                        nc.sync.dma_start(b1r[:], bn1b_d[0:1, sl])
                        glob_j = bn1_io[j][1]
                        sc, bi = norm_params_chunk(
                            pT, glob_j[:, 0:CH], glob_j[:, CH:2 * CH],
                            NBC, f"bn1_{j}", g_row_sl=g1r[:], b_row_sl=b1r[:],
                            sc_tag="bnsc", bi_tag="bnbi", bc_bufs=1)
                        for k in range(CT):
                            affine_chunk(u1[k][:, sl], u1[k][:, sl],
                                         sc[:], bi[:])

                    def bn2_params(i, last=False):
                        """BN2 normalize params for sub-chunk i."""
                        off, width = FFN_SUB[i]
                        slw = slice(off, off + width)
                        g2r = pT.tile([1, CH], fp32, tag="rg", bufs=1,
                                      name=f"bn2g_{i}")
                        nc.sync.dma_start(g2r[:, :width], bn2g_d[0:1, slw])
                        b2r = pT.tile([1, CH], fp32, tag="rgb", bufs=1,
                                      name=f"bn2b_{i}")
                        nc.sync.dma_start(b2r[:, :width], bn2b_d[0:1, slw])
                        glob_i = bn2_io[i][1]
                        return norm_params_chunk(
                            pT, glob_i[:, 0:width], glob_i[:, width:2 * width],
                            NBC, f"bn2_{i}", g_row_sl=g2r[:, :width],
                            b_row_sl=b2r[:, :width], sc_tag="nsc",
                            bi_tag="nbi", bc_bufs=1, via_dma=not last,
                            width=width)

                    def bn2_finale_k(i, k, sc2, bi2):
                        off, width = FFN_SUB[i]
                        slw = slice(off, off + width)
                        yk = pT.tile([P, CH], fp32, tag="yout", bufs=2,
                                     name=f"y_{k}_{i}")
                        nc.vector.tensor_mul(yk[:, :width], u1[k][:, slw],
                                             sc2[:, :width])
                        nc.vector.tensor_add(yk[:, :width], yk[:, :width],
                                             bi2[:, :width])
                        nc.sync.dma_start(yT_d[ts(k, P), slw], yk[:, :width])

                    pp = None
                    for i, (off, width) in enumerate(FFN_SUB):
                        slw = slice(off, off + width)
                        # ---- FFN mm1 ----
                        z = []
                        for f in range(FT):
                            w1f = wstream.tile([P, CT, P], bf16, tag="w1f",
                                               bufs=2, name=f"w1f_{i}_{f}")
                            nc.sync.dma_start(w1f[:], w1_d[f])
                            zps = ppw.tile([P, CH], fp32, tag="w",
                                           name=f"z_ps_{i}_{f}")
                            for k in range(CT):
                                nc.tensor.matmul(zps[:, :width], w1f[:, k, :],
                                                 h2T[k][:, slw],
                                                 start=(k == 0),
                                                 stop=(k == CT - 1))
                            zf = p6.tile([P, CH], bf16, tag=f"z{f}",
                                         name=f"z_{i}_{f}")
                            nc.scalar.activation(zf[:, :width], zps[:, :width],
                                                 AF.Relu,
                                                 bias=b1_sb[:, f:f + 1],
                                                 scale=1.0)
                            z.append(zf)
                        # params consumed by this / the previous sub-chunk's
                        # mm2-side work; their collectives are long done
                        if i == 0:
                            ln2_apply(1)
                            bn1_globals(0)
                        elif i == 1:
                            bn1_globals(1)
                        if i > 0:
                            pp = bn2_params(i - 1)
                        # ---- FFN mm2 + residual + BN2 stats ----
                        for k in range(CT):
                            w2k = p6.tile([P, FT, P], bf16, tag="w2k", bufs=2,
                                          name=f"w2k_{i}_{k}")
                            nc.sync.dma_start(w2k[:], w2_d[k])
                            yps = ppw.tile([P, CH], fp32, tag="w",
                                           name=f"y_ps_{i}_{k}")
                            for f in range(FT):
                                nc.tensor.matmul(yps[:, :width], w2k[:, f, :],
                                                 z[f][:, :width],
                                                 start=(f == 0),
                                                 stop=(f == FT - 1))
                            nc.vector.scalar_tensor_tensor(
                                out=u1[k][:, slw], in0=yps[:, :width],
                                scalar=b2_sb[:, k:k + 1], in1=u1[k][:, slw],
                                op0=OP.add, op1=OP.add)
                            stats_chunk(u1[k][:, slw], stp_bn2[i],
                                        k == 0, k == CT - 1, width=width)
                            if i > 0:
                                bn2_finale_k(i - 1, k, pp[0], pp[1])
                        bn2_io[i] = allreduce_chunk(u1pool, stp_bn2[i],
                                                    f"bn2_{i}", width=width)
                    # last sub-chunk: PE is idle, use the low-latency matmul
                    # broadcast instead of the DMA round trip
                    pp = bn2_params(NS - 1, last=True)
                    for k in range(CT):
                        bn2_finale_k(NS - 1, k, pp[0], pp[1])
              h2pool.release()

    nc.compile()
    return nc


def _get_program():
    global _PROG
    if _PROG is None:
        _PROG = _build()
    return _PROG


def _prep_shared(inputs):
    """Host-side weight folding + pre-tiling; identical for every core."""
    f32 = np.float32
    bf16 = ml_dtypes.bfloat16
    wq = np.asarray(inputs["wq"], f32)      # (H, C, D)
    wk = np.asarray(inputs["wk"], f32)
    wv = np.asarray(inputs["wv"], f32)
    wo = np.asarray(inputs["wo"], f32)      # (C, C)
    bo = np.asarray(inputs["bo"], f32)      # (C,)
    g1 = np.asarray(inputs["ln1_g"], f32)
    b1n = np.asarray(inputs["ln1_b"], f32)
    g2 = np.asarray(inputs["ln2_g"], f32)
    b2n = np.asarray(inputs["ln2_b"], f32)
    w1 = np.asarray(inputs["w1"], f32)      # (C, F)
    b1 = np.asarray(inputs["b1"], f32)      # (F,)
    w2 = np.asarray(inputs["w2"], f32)      # (F, C)
    b2 = np.asarray(inputs["b2"], f32)      # (C,)

    dscale = f32(D) ** f32(-0.5)
    # fold ln1 affine into qkv projections; q also takes 1/sqrt(D)
    wq2 = (wq * g1[None, :, None] * dscale).transpose(1, 0, 2).reshape(C, C)
    wk2 = (wk * g1[None, :, None]).transpose(1, 0, 2).reshape(C, C)
    wv2 = (wv * g1[None, :, None]).transpose(1, 0, 2).reshape(C, C)
    bq = (np.einsum("c,hcd->hd", b1n, wq) * dscale).reshape(C)
    bv = np.einsum("c,hcd->hd", b1n, wv).reshape(C)
    # k-side bias cancels in softmax (constant per row); v bias folds into bo
    bo2 = bo + bv @ wo
    w1f = g2[:, None] * w1
    b1f = b1 + b2n @ w1

    def lhst_tiles(w, n_out):
        # (C_in, n_out*P) -> (n_out, P, C_in//P, P):
        # [o, p, ki, n] = w[ki*P + p, o*P + n]
        ci = w.shape[0]
        return np.ascontiguousarray(
            w.reshape(ci // P, P, n_out, P).transpose(2, 1, 0, 3)
        ).astype(bf16)

    def cols(v, n):  # (n*P,) -> (P, n) with [p, i] = v[i*P + p]
        return np.ascontiguousarray(v.reshape(n, P).T, dtype=f32)

    def row(v):
        return np.ascontiguousarray(v.reshape(1, T), dtype=f32)

    return dict(
        wq=lhst_tiles(wq2, H), wk=lhst_tiles(wk2, H),
        wv=wv2.astype(bf16),
        bq=cols(bq, H), wo=lhst_tiles(wo, CT), bo=cols(bo2, CT),
        w1=lhst_tiles(w1f, FT), b1=cols(b1f, FT),
        w2=lhst_tiles(w2, CT), b2=cols(b2, CT),
        bn1g=row(np.asarray(inputs["bn1_g"], f32)),
        bn1b=row(np.asarray(inputs["bn1_b"], f32)),
        bn2g=row(np.asarray(inputs["bn2_g"], f32)),
        bn2b=row(np.asarray(inputs["bn2_b"], f32)),
    )


def _run(inputs, trace=False):
    from concourse import bass_utils
    nc = _get_program()
    x = np.asarray(inputs["x"], np.float32)
    shared = _prep_shared(inputs)
    in_maps = []
    for b in range(B):
        m = dict(shared)
        xt = np.ascontiguousarray(x[b].T)
        m["xT"] = xt
        m["xbf"] = xt.astype(ml_dtypes.bfloat16)
        in_maps.append(m)
    res = bass_utils.run_bass_kernel_spmd(
        nc, in_maps, core_ids=list(range(NCORES)), trace=trace)
    out = np.stack([res.results[b]["yT"].T for b in range(B)]).astype(np.float32)
    return out, res


def kernel(**inputs):
    out, _ = _run(inputs, trace=False)
    return out
